# revision 47
# baseline (speedup 1.0000x reference)
"""Trainium2 Bass kernel for the Capsule routing layer (nn_Capsule_49658411876931).

Math (see reference):
    u_hat[b,j,i,d] = sum_k W[j,i,d,k] * x[b,i,k]
    b0 = 0
    for r in 0..2:
        c = softmax(b, axis=j)
        s[b,j,d] = sum_i c[b,j,i] u_hat[b,j,i,d]
        v = squash(s)  (over d)
        if r < 2: b += sum_d u_hat[b,j,i,d] v[b,j,d]
    return v  [B, J, D]

Sharding: batch B=32 split over 8 cores (B_LOC=4); W replicated (16.8 MB
bf16 per core, streamed once from HBM in 1 MB chunks at ~340 GB/s). The
routing loop is then fully core-local: no collectives at all (the previous
I-sharded design lost ~90 us to three ncfw AllReduce latencies).

Per-core layouts (P = SBUF partition index):
  i = ((g*2 + rp)*4 + c)*8 + i8   (g in 0..31, rp in 0..1, c in 0..3, i8 in 0..7)
  u_hat "C" tensor : [P = 32c + 4*i8 + b, free = (g, rp, d, j)]  bf16
  logits / c       : [P = 32c + 4*i8 + b, free = (g, rp, j)]
u_hat is computed with 8-way tile_position-packed PE matmuls: stationary is a
host-prepped block-diagonal x chunk [64 rows = (i8,k), 32 cols = (i8',b)]
(zero off-diagonal), moving is the W chunk [64, 512]; each matmul covers 8
input capsules, 8 matmuls (rp, c) run concurrently per round g.

s[b,dj] = sum_i c*u_hat runs on the PE as 64 accumulating matmuls against a
"collapse the 32 (c,i8)-strips" selector stationary (delta_{p%4,b}); for
iteration 0 the softmax coefficients are the constant 1/J, folded into a
(1/J)-scaled selector so no elementwise pass is needed. In iterations 1-2
the 1/sum_j exp softmax divisor is folded into the selector as well
(per-partition-and-slice scale), skipping the c = p_t/S elementwise pass.
The agreement d-contraction runs as a log2 tree of bf16 2x-mode
tensor_tensor adds. The j-softmax is slice-local, so each routing block
runs its full chain (agreement tree -> logits -> exp -> block softmax ->
coefficient product -> s-matmuls) before the next block: the 13.8 us
serial s-matmul chain on the PE overlaps the remaining blocks' DVE work.
squash's rsqrt is a 2-step Newton iteration on the DVE (quake bitcast
seed), so the ACT engine never leaves the exp table set (a Sqrt/Ln
activation would reload tables ~2.6 us per iteration; measured).

Rejected with evidence: fp8e4 W (rel err 2.47e-2 > the 2e-2 gate, scale-
invariant - pure 3-bit-mantissa error), GpSimd elementwise offload (~6.5us
per 512-el/partition tensor_tensor vs 0.6us on DVE - dispatch dominated),
single-ring or SWDGE W streaming (275 GB/s vs 323 on alternating HWDGE
rings), 2 MB W chunks with prefetch depth 1 (+9 us).
"""

import numpy as np
import ml_dtypes

import concourse.bass as bass
import concourse.tile as tile
from concourse import bacc, mybir
from concourse.bass_utils import run_bass_kernel_spmd

F32 = mybir.dt.float32
BF16 = mybir.dt.bfloat16
FP8 = mybir.dt.float8e4
Alu = mybir.AluOpType
Act = mybir.ActivationFunctionType

B, I, K = 32, 2048, 8
J, D = 32, 16
JD = J * D                     # 512
NCORES = 8
B_LOC = B // NCORES            # 4
NG = 32                        # rounds g; i = ((g*2+rp)*4+c)*8 + i8
CHUNK_G = 4                    # g's per W DMA chunk (2 MB bf16)
NCHUNK = NG // CHUNK_G         # 8
W_FP8 = False                  # fp8 W fails the 2e-2 gate (measured 2.5e-2)
W_SCALE = 64.0                 # power-of-2, folded exactly into bf16 xs
GB = 4                         # g's per routing block
NBLK = NG // GB                # 8 routing blocks (8 (g,rp)-slices each)
ROUTINGS = 3
EPS = 1e-7

_CACHE = {}


def _build():
    nc = bacc.Bacc("TRN2", target_bir_lowering=False, debug=False, num_devices=NCORES)

    wt_in = nc.dram_tensor(
        "wt", [NCHUNK, 128, CHUNK_G, 4, JD], FP8 if W_FP8 else BF16,
        kind="ExternalInput",
    )
    xs_in = nc.dram_tensor("xs", [128, NG, 4, 32], BF16, kind="ExternalInput")
    v_out = nc.dram_tensor("v", [B_LOC, J, D], F32, kind="ExternalOutput")

    # Selector constants for cross-partition PE ops:
    #   sel[p, b'] = 1 iff p % 4 == b'     (collapse the 32 (c,i8)-strips)
    #   sel32 = sel / J                    (fold in the uniform iter-0 softmax)
    #   selT[b, p] = sel^T                 (replicate over the 32 strips)
    p_idx = np.arange(128)
    sel_np = (p_idx[:, None] % B_LOC == np.arange(B_LOC)[None, :]).astype(np.float32)
    selpack = np.zeros((128, 2 * B_LOC + 128), np.float32)
    selpack[:, 0:B_LOC] = sel_np
    selpack[:, B_LOC : 2 * B_LOC] = sel_np / J
    selpack[0:B_LOC, 2 * B_LOC :] = sel_np.T
    sel_dram = nc.inline_tensor(selpack.astype(ml_dtypes.bfloat16), "selpack")

    with tile.TileContext(nc) as tc:
        with (
            tc.tile_pool(name="persist", bufs=1) as pp,
            tc.tile_pool(name="small", bufs=1) as sp,
            tc.tile_pool(name="spsum", bufs=1, space="PSUM") as ssp,
        ):
            # ---- persistent SBUF tensors ----
            xs = pp.tile([128, NG, 4, 32], BF16)        # block-diag x stationaries
            C = pp.tile([128, NG, 2, D, J], BF16)       # u_hat
            bl = pp.tile([128, NG, 2, J], F32)          # routing logits
            p_t = pp.tile([128, NG, 2, J], BF16)        # exp(b)
            selc = pp.tile([128, 2 * B_LOC + 128], BF16)
            v_rep = pp.tile([128, D, J], BF16)          # v replicated over strips

            sel = selc[:, 0:B_LOC]
            sel32 = selc[:, B_LOC : 2 * B_LOC]
            selT = selc[0:B_LOC, 2 * B_LOC :]

            # persistent PSUM: s accumulator + v/fac replication banks
            s_ps = ssp.tile([B_LOC, D * J], F32)
            vr_ps = ssp.tile([128, D * J], F32)
            fr_ps = ssp.tile([128, J], F32)

            # a tiny leading xs slice (g 0-1, 64 KB) unblocks the first
            # matmuls ~3us earlier than a monolithic xs transfer; the rest
            # streams behind it. selc trails (only needed once s0 starts).
            # No bl memset: iteration 0's logit update writes bl.
            for qs in (slice(0, 2), slice(2, 8), slice(8, 16),
                       slice(16, 24), slice(24, 32)):
                nc.gpsimd.dma_start(xs[:, qs], xs_in[:, qs])
            nc.scalar.dma_start(selc[:], sel_dram[:])

            # Warm the ACT exp table set at t~0 (under the W DMA shadow) so
            # the first softmax doesn't stall ~2.7us. The squash sqrt runs as
            # a DVE Newton rsqrt, so exp is the only table set ever loaded.
            wa = sp.tile([1, 8], F32, tag="wa")
            wb = sp.tile([1, 8], F32, tag="wb")
            nc.vector.memset(wa[:], 1.0)
            nc.scalar.activation(wb[:], wa[:], Act.Exp)

            # constants for the Newton rsqrt (quake seed)
            U32 = mybir.dt.uint32
            rc_magic = sp.tile([B_LOC, 1], U32, tag="rc_magic")
            rc_one = sp.tile([B_LOC, 1], U32, tag="rc_one")
            rc_15 = sp.tile([B_LOC, 1], F32, tag="rc_15")
            nc.vector.memset(rc_magic[:], 0x5F3759DF)
            nc.vector.memset(rc_one[:], 1)
            nc.vector.memset(rc_15[:], 1.5)

            # ---- phase 1: u_hat + iteration-0 s accumulation ----
            ns0 = [0]

            def s0_slice(g, rp):
                kk = ns0[0]
                ns0[0] += 1
                nc.tensor.matmul(
                    s_ps[:],
                    sel32,
                    C[:, g, rp].rearrange("p d j -> p (d j)"),
                    start=(kk == 0),
                    stop=(kk == 2 * NG - 1),
                )

            with (
                tc.tile_pool(name="wpool", bufs=3) as wp,
                tc.tile_pool(name="psum1", bufs=2, space="PSUM") as ps1,
            ):
                def w_dma(ch):
                    wt = wp.tile([128, CHUNK_G, 4, JD], FP8 if W_FP8 else BF16, tag="wt")
                    # alternate the two HWDGE rings so two chunks are always in
                    # flight; all PSUM->SBUF casts run on Vector so the scalar
                    # ring's W issues are never queued behind a copy
                    eng = nc.sync if ch % 2 == 0 else nc.scalar
                    eng.dma_start(wt[:], wt_in[ch])
                    return wt

                wts = [w_dma(0), w_dma(1)]
                nsub = 0
                for ch in range(NCHUNK):
                    wt = wts.pop(0)
                    if ch + 2 < NCHUNK:
                        wts.append(w_dma(ch + 2))
                    for gg in range(CHUNK_G):
                        g = ch * CHUNK_G + gg
                        pg = ps1.tile([128, 2, JD], F32, tag="pg")
                        for rp in range(2):
                            for c in range(4):
                                nc.tensor.matmul(
                                    pg[32 * c : 32 * c + 32, rp, :],
                                    xs[64 * rp : 64 * rp + 64, g, c, :],
                                    wt[64 * rp : 64 * rp + 64, gg, c, :],
                                    tile_position=(64 * rp, 32 * c),
                                )
                        dst = C[:, g].rearrange("p rp d j -> p rp (d j)")
                        # tail copies go to ACT: by then every W chunk has
                        # been issued, so the scalar ring has nothing to delay
                        if g >= NG - 6 and g % 2 == 0:
                            nc.scalar.copy(dst, pg[:])
                        else:
                            nc.vector.tensor_copy(dst, pg[:])
                        nsub += 1
                        if nsub > 2:
                            g2 = nsub - 3
                            s0_slice(g2, 0)
                            s0_slice(g2, 1)
                for g2 in range(NG - 2, NG):
                    s0_slice(g2, 0)
                    s0_slice(g2, 1)

            # ---- routing ----
            with tc.tile_pool(name="blk", bufs=2) as bp:
                for it in range(ROUTINGS):
                    # squash factor from s_ps (all on [B_LOC(=4), ...]);
                    # Square first: the n2 chain is the critical path, the
                    # s_gb copy only feeds the later replication matmul
                    sq = sp.tile([B_LOC, D, J], F32, tag="sq")
                    nc.scalar.activation(
                        sq.rearrange("b d j -> b (d j)"), s_ps[:], Act.Square
                    )
                    s_gb = sp.tile([B_LOC, D, J], BF16, tag="s_gb")
                    nc.scalar.copy(s_gb.rearrange("b d j -> b (d j)"), s_ps[:])
                    n2 = sp.tile([B_LOC, J], F32, tag="n2")
                    nc.vector.tensor_reduce(
                        n2[:],
                        sq.rearrange("b d j -> b j d"),
                        axis=mybir.AxisListType.X,
                        op=Alu.add,
                    )
                    # factor = n2 / (1 + n2) / sqrt(n2 + eps); rsqrt via a
                    # 2-step Newton iteration with the quake bitcast seed —
                    # all DVE, so the ACT engine never leaves the exp table
                    # set (a Sqrt/Ln activation would reload tables ~2.6us
                    # per iteration)
                    n2e = sp.tile([B_LOC, J], F32, tag="n2e")
                    nc.vector.tensor_scalar_add(n2e[:], n2[:], EPS)
                    hf = sp.tile([B_LOC, J], F32, tag="hf")
                    nc.vector.tensor_scalar_mul(hf[:], n2e[:], 0.5)
                    zi = sp.tile([B_LOC, J], U32, tag="zi")
                    nc.vector.tensor_tensor(
                        zi[:], n2e.bitcast(U32),
                        rc_one.broadcast_to([B_LOC, J]),
                        op=Alu.logical_shift_right,
                    )
                    ry = sp.tile([B_LOC, J], F32, tag="ry")
                    nc.vector.tensor_tensor(
                        ry.bitcast(U32), rc_magic.broadcast_to([B_LOC, J]),
                        zi[:], op=Alu.subtract,
                    )
                    rt = sp.tile([B_LOC, J], F32, tag="rt")
                    for _ in range(2):
                        nc.vector.tensor_tensor(rt[:], ry[:], ry[:], op=Alu.mult)
                        nc.vector.tensor_tensor(rt[:], rt[:], hf[:], op=Alu.mult)
                        nc.vector.tensor_tensor(
                            rt[:], rc_15.broadcast_to([B_LOC, J]), rt[:],
                            op=Alu.subtract,
                        )
                        nc.vector.tensor_tensor(ry[:], ry[:], rt[:], op=Alu.mult)
                    tmp = sp.tile([B_LOC, J], F32, tag="tmp")
                    nc.vector.tensor_scalar_add(tmp[:], n2[:], 1.0)
                    qr = sp.tile([B_LOC, J], F32, tag="qr")
                    nc.vector.reciprocal(qr[:], tmp[:])
                    fac = sp.tile([B_LOC, J], F32, tag="fac")
                    nc.vector.tensor_tensor(fac[:], qr[:], ry[:], op=Alu.mult)

                    if it < ROUTINGS - 1:
                        facb = sp.tile([B_LOC, J], BF16, tag="facb")
                        nc.vector.tensor_tensor(facb[:], fac[:], n2[:], op=Alu.mult)
                        # replicate s and fac over the 32 strips via the PE
                        nc.tensor.matmul(
                            vr_ps[:], selT, s_gb.rearrange("b d j -> b (d j)")
                        )
                        nc.tensor.matmul(fr_ps[:], selT, facb[:])
                        fr_sb = sp.tile([128, J], BF16, tag="fr_sb")
                        nc.scalar.copy(fr_sb[:], fr_ps[:])
                        nc.vector.tensor_tensor(
                            v_rep[:],
                            vr_ps.rearrange("p (d j) -> p d j", d=D, j=J),
                            fr_sb[:, None, :].broadcast_to([128, D, J]),
                            op=Alu.mult,
                        )
                        # fused per-block routing step. The j-softmax is
                        # slice-local, so each block runs its ENTIRE chain —
                        # agreement (log2 tree over d), logit update, exp,
                        # block-local softmax sum (folded into the s-matmul
                        # selector), coefficient product, s-matmuls — before
                        # the next block. The s-matmul chain (13.8 us serial
                        # on the PE) then overlaps the remaining blocks' DVE
                        # work instead of all queueing after pass A.
                        for blk in range(NBLK):
                            gs = slice(blk * GB, blk * GB + GB)
                            Cb = C[:, gs]
                            pi2 = bp.tile([128, GB, 2, D, J], BF16, tag="pi2")
                            nc.vector.tensor_tensor(
                                pi2[:],
                                Cb,
                                v_rep[:, None, None, :, :].broadcast_to(
                                    [128, GB, 2, D, J]
                                ),
                                op=Alu.mult,
                            )
                            t8 = bp.tile([128, GB, 2, 8, J], BF16, tag="t8")
                            nc.vector.tensor_tensor(
                                t8[:], pi2[:, :, :, 0:8, :], pi2[:, :, :, 8:16, :],
                                op=Alu.add,
                            )
                            t4 = bp.tile([128, GB, 2, 4, J], BF16, tag="t4")
                            nc.vector.tensor_tensor(
                                t4[:], t8[:, :, :, 0:4, :], t8[:, :, :, 4:8, :],
                                op=Alu.add,
                            )
                            t2 = bp.tile([128, GB, 2, 2, J], BF16, tag="t2")
                            nc.vector.tensor_tensor(
                                t2[:], t4[:, :, :, 0:2, :], t4[:, :, :, 2:4, :],
                                op=Alu.add,
                            )
                            t1 = bp.tile([128, GB, 2, J], BF16, tag="t1")
                            nc.vector.tensor_tensor(
                                t1[:], t2[:, :, :, 0, :], t2[:, :, :, 1, :],
                                op=Alu.add,
                            )
                            if it == 0:
                                nc.vector.tensor_copy(bl[:, gs], t1[:])
                            else:
                                nc.vector.tensor_add(bl[:, gs], bl[:, gs], t1[:])
                            nc.scalar.activation(p_t[:, gs], bl[:, gs], Act.Exp)
                            S = bp.tile([128, GB, 2], F32, tag="S")
                            nc.vector.tensor_reduce(
                                S[:], p_t[:, gs], axis=mybir.AxisListType.X,
                                op=Alu.add,
                            )
                            Sr = bp.tile([128, GB, 2], BF16, tag="Sr")
                            with nc.allow_low_precision(
                                reason="softmax divisor in bf16: c is consumed "
                                "in bf16 products anyway"
                            ):
                                nc.vector.reciprocal(Sr[:], S[:])
                            sel_s = bp.tile([128, GB, 2, B_LOC], BF16, tag="sel_s")
                            nc.vector.tensor_tensor(
                                sel_s[:],
                                sel[:, None, None, :].broadcast_to(
                                    [128, GB, 2, B_LOC]
                                ),
                                Sr[:, :, :, None].broadcast_to(
                                    [128, GB, 2, B_LOC]
                                ),
                                op=Alu.mult,
                            )
                            nsplit = 2 if blk == NBLK - 1 else 1
                            for sb in range(nsplit):
                                gw = GB // nsplit
                                goff = blk * GB + sb * gw
                                gsp = slice(goff, goff + gw)
                                pi = bp.tile([128, GB, 2, D, J], BF16, tag="pi")
                                piv = pi[:, 0:gw]
                                nc.vector.tensor_tensor(
                                    piv,
                                    C[:, gsp],
                                    p_t[:, gsp, :, None, :].broadcast_to(
                                        [128, gw, 2, D, J]
                                    ),
                                    op=Alu.mult,
                                )
                                for gr in range(gw * 2):
                                    g2, rp = divmod(gr, 2)
                                    kk = goff * 2 + gr
                                    nc.tensor.matmul(
                                        s_ps[:],
                                        sel_s[:, sb * gw + g2, rp, :],
                                        piv[:, g2, rp].rearrange(
                                            "p d j -> p (d j)"
                                        ),
                                        start=(kk == 0),
                                        stop=(kk == 2 * NG - 1),
                                    )
                    else:
                        # final output: v = s * fac in f32, (d, j) -> (j, d)
                        facf = sp.tile([B_LOC, J], F32, tag="facf")
                        nc.vector.tensor_tensor(
                            facf[:], fac[:], n2[:], op=Alu.mult
                        )
                        v_jd = sp.tile([B_LOC, J, D], F32, tag="v_jd")
                        nc.vector.tensor_tensor(
                            v_jd[:],
                            s_gb.rearrange("b d j -> b j d"),
                            facf[:, :, None].broadcast_to([B_LOC, J, D]),
                            op=Alu.mult,
                        )
                        nc.sync.dma_start(v_out[:], v_jd[:])

    nc.compile()
    return nc


def _prep_inputs(x, W):
    """Host-side layout prep (bf16). W is shared by all cores; x is B-sliced."""
    # wt[ch, p=(rp,i8,k), gg, c, (d j)] = W[j, i, d, k],
    # i = ((g*2+rp)*4+c)*8 + i8, g = ch*CHUNK_G + gg
    Wr = W.reshape(J, NG, 2, 4, 8, D, K)                  # j g rp c i8 d k
    wt = np.ascontiguousarray(Wr.transpose(1, 2, 4, 6, 3, 5, 0))  # g rp i8 k c d j
    wt = wt.reshape(NCHUNK, CHUNK_G, 128, 4, JD).transpose(0, 2, 1, 3, 4)
    if W_FP8:
        wt = np.ascontiguousarray(wt * W_SCALE).astype(ml_dtypes.float8_e4m3)
    else:
        wt = np.ascontiguousarray(wt).astype(ml_dtypes.bfloat16)
    i8 = np.arange(8)
    if W_FP8:
        x = x / W_SCALE  # exact in bf16 (power-of-2); undoes the W scale
    in_maps = []
    for m in range(NCORES):
        xb = x[B_LOC * m : B_LOC * (m + 1)].reshape(B_LOC, NG, 2, 4, 8, K)
        xsz = np.zeros((2, 8, K, NG, 4, 8, B_LOC), np.float32)  # rp i8 k g c i8' b
        xsz[:, i8, :, :, :, i8, :] = xb.transpose(4, 2, 5, 1, 3, 0)[i8]
        in_maps.append(
            {
                "wt": wt,
                "xs": xsz.reshape(128, NG, 4, 32).astype(ml_dtypes.bfloat16),
            }
        )
    return in_maps


def run(inputs, trace=False):
    if "nc" not in _CACHE:
        _CACHE["nc"] = _build()
    nc = _CACHE["nc"]
    in_maps = _prep_inputs(np.asarray(inputs["x"]), np.asarray(inputs["W"]))
    bkr = run_bass_kernel_spmd(
        nc, in_maps, core_ids=list(range(NCORES)), trace=trace
    )
    out = np.concatenate(
        [bkr.results[m]["v"].astype(np.float32) for m in range(NCORES)], axis=0
    )
    return out, bkr


def kernel(x, W):
    out, _ = run({"x": np.asarray(x), "W": np.asarray(W)})
    return out


# revision 48
# speedup vs baseline: 1.1924x; 1.1924x over previous
"""Trainium2 Bass kernel for the Capsule routing layer (nn_Capsule_49658411876931).

Math (see reference):
    u_hat[b,j,i,d] = sum_k W[j,i,d,k] * x[b,i,k]
    b0 = 0
    for r in 0..2:
        c = softmax(b, axis=j)
        s[b,j,d] = sum_i c[b,j,i] u_hat[b,j,i,d]
        v = squash(s)  (over d)
        if r < 2: b += sum_d u_hat[b,j,i,d] v[b,j,d]
    return v  [B, J, D]

Sharding: batch B=32 split over 8 cores (B_LOC=4); W replicated (16.8 MB
bf16 per core, streamed once from HBM in 1 MB chunks at ~340 GB/s). The
routing loop is then fully core-local: no collectives at all (the previous
I-sharded design lost ~90 us to three ncfw AllReduce latencies).

Per-core layouts (P = SBUF partition index):
  i = ((g*2 + rp)*4 + c)*8 + i8   (g in 0..31, rp in 0..1, c in 0..3, i8 in 0..7)
  u_hat "C" tensor : [P = 32c + 4*i8 + b, free = (g, rp, d, j)]  bf16
  logits / c       : [P = 32c + 4*i8 + b, free = (g, rp, j)]
u_hat is computed with 8-way tile_position-packed PE matmuls: stationary is a
host-prepped block-diagonal x chunk [64 rows = (i8,k), 32 cols = (i8',b)]
(zero off-diagonal), moving is the W chunk [64, 512]; each matmul covers 8
input capsules, 8 matmuls (rp, c) run concurrently per round g.

s[b,dj] = sum_i c*u_hat runs on the PE as 64 accumulating matmuls against a
"collapse the 32 (c,i8)-strips" selector stationary (delta_{p%4,b}); for
iteration 0 the softmax coefficients are the constant 1/J, folded into a
(1/J)-scaled selector so no elementwise pass is needed. In iterations 1-2
the 1/sum_j exp softmax divisor is folded into the selector as well
(per-partition-and-slice scale), skipping the c = p_t/S elementwise pass.
The agreement d-contraction runs as a log2 tree of bf16 2x-mode
tensor_tensor adds. The j-softmax is slice-local, so each routing block
runs its full chain (agreement tree -> logits -> exp -> block softmax ->
coefficient product -> s-matmuls) before the next block: the 13.8 us
serial s-matmul chain on the PE overlaps the remaining blocks' DVE work.
squash's rsqrt is a 2-step Newton iteration on the DVE (quake bitcast
seed), so the ACT engine never leaves the exp table set (a Sqrt/Ln
activation would reload tables ~2.6 us per iteration; measured).

Rejected with evidence: fp8e4 W (rel err 2.47e-2 > the 2e-2 gate, scale-
invariant - pure 3-bit-mantissa error), GpSimd elementwise offload (~6.5us
per 512-el/partition tensor_tensor vs 0.6us on DVE - dispatch dominated),
single-ring or SWDGE W streaming (275 GB/s vs 323 on alternating HWDGE
rings), 2 MB W chunks with prefetch depth 1 (+9 us).
"""

import numpy as np
import ml_dtypes

import concourse.bass as bass
import concourse.tile as tile
from concourse import bacc, mybir
from concourse.bass_utils import run_bass_kernel_spmd

F32 = mybir.dt.float32
BF16 = mybir.dt.bfloat16
FP8 = mybir.dt.float8e4
Alu = mybir.AluOpType
Act = mybir.ActivationFunctionType

B, I, K = 32, 2048, 8
J, D = 32, 16
JD = J * D                     # 512
NCORES = 8
B_LOC = B // NCORES            # 4
NG = 32                        # rounds g; i = ((g*2+rp)*4+c)*8 + i8
CHUNK_G = 2                    # g's per W DMA chunk (1 MB bf16)
NCHUNK = NG // CHUNK_G         # 16
W_FP8 = False                  # fp8 W fails the 2e-2 gate (measured 2.5e-2)
W_SCALE = 64.0                 # power-of-2, folded exactly into bf16 xs
GB = 4                         # g's per routing block
NBLK = NG // GB                # 8 routing blocks (8 (g,rp)-slices each)
ROUTINGS = 3
EPS = 1e-7

_CACHE = {}


def _build():
    nc = bacc.Bacc("TRN2", target_bir_lowering=False, debug=False, num_devices=NCORES)

    wt_in = nc.dram_tensor(
        "wt", [NCHUNK, 128, CHUNK_G, 4, JD], FP8 if W_FP8 else BF16,
        kind="ExternalInput",
    )
    xs_in = nc.dram_tensor("xs", [128, NG, 4, 32], BF16, kind="ExternalInput")
    v_out = nc.dram_tensor("v", [B_LOC, J, D], F32, kind="ExternalOutput")

    # Selector constants for cross-partition PE ops:
    #   sel[p, b'] = 1 iff p % 4 == b'     (collapse the 32 (c,i8)-strips)
    #   sel32 = sel / J                    (fold in the uniform iter-0 softmax)
    #   selT[b, p] = sel^T                 (replicate over the 32 strips)
    p_idx = np.arange(128)
    sel_np = (p_idx[:, None] % B_LOC == np.arange(B_LOC)[None, :]).astype(np.float32)
    selpack = np.zeros((128, 2 * B_LOC + 128), np.float32)
    selpack[:, 0:B_LOC] = sel_np
    selpack[:, B_LOC : 2 * B_LOC] = sel_np / J
    selpack[0:B_LOC, 2 * B_LOC :] = sel_np.T
    sel_dram = nc.inline_tensor(selpack.astype(ml_dtypes.bfloat16), "selpack")

    with tile.TileContext(nc) as tc:
        with (
            tc.tile_pool(name="persist", bufs=1) as pp,
            tc.tile_pool(name="small", bufs=1) as sp,
            tc.tile_pool(name="spsum", bufs=1, space="PSUM") as ssp,
        ):
            # ---- persistent SBUF tensors ----
            xs = pp.tile([128, NG, 4, 32], BF16)        # block-diag x stationaries
            C = pp.tile([128, NG, 2, D, J], BF16)       # u_hat
            bl = pp.tile([128, NG, 2, J], F32)          # routing logits
            p_t = pp.tile([128, NG, 2, J], BF16)        # exp(b)
            selc = pp.tile([128, 2 * B_LOC + 128], BF16)
            v_rep = pp.tile([128, D, J], BF16)          # v replicated over strips

            sel = selc[:, 0:B_LOC]
            sel32 = selc[:, B_LOC : 2 * B_LOC]
            selT = selc[0:B_LOC, 2 * B_LOC :]

            # persistent PSUM: s accumulator + v/fac replication banks
            s_ps = ssp.tile([B_LOC, D * J], F32)
            vr_ps = ssp.tile([128, D * J], F32)
            fr_ps = ssp.tile([128, J], F32)

            # a tiny leading xs slice (g 0-1, 64 KB) unblocks the first
            # matmuls ~3us earlier than a monolithic xs transfer; the rest
            # streams behind it. selc trails (only needed once s0 starts).
            # No bl memset: iteration 0's logit update writes bl.
            for qs in (slice(0, 2), slice(2, 8), slice(8, 16),
                       slice(16, 24), slice(24, 32)):
                nc.gpsimd.dma_start(xs[:, qs], xs_in[:, qs])
            nc.scalar.dma_start(selc[:], sel_dram[:])

            # Warm the ACT exp table set at t~0 (under the W DMA shadow) so
            # the first softmax doesn't stall ~2.7us. The squash sqrt runs as
            # a DVE Newton rsqrt, so exp is the only table set ever loaded.
            wa = sp.tile([1, 8], F32, tag="wa")
            wb = sp.tile([1, 8], F32, tag="wb")
            nc.vector.memset(wa[:], 1.0)
            nc.scalar.activation(wb[:], wa[:], Act.Exp)

            # constants for the Newton rsqrt (quake seed)
            U32 = mybir.dt.uint32
            rc_magic = sp.tile([B_LOC, 1], U32, tag="rc_magic")
            rc_one = sp.tile([B_LOC, 1], U32, tag="rc_one")
            rc_15 = sp.tile([B_LOC, 1], F32, tag="rc_15")
            nc.vector.memset(rc_magic[:], 0x5F3759DF)
            nc.vector.memset(rc_one[:], 1)
            nc.vector.memset(rc_15[:], 1.5)

            # ---- phase 1: u_hat + iteration-0 s accumulation ----
            ns0 = [0]

            def s0_slice(g, rp):
                kk = ns0[0]
                ns0[0] += 1
                nc.tensor.matmul(
                    s_ps[:],
                    sel32,
                    C[:, g, rp].rearrange("p d j -> p (d j)"),
                    start=(kk == 0),
                    stop=(kk == 2 * NG - 1),
                )

            with (
                tc.tile_pool(name="wpool", bufs=5) as wp,
                tc.tile_pool(name="psum1", bufs=2, space="PSUM") as ps1,
            ):
                def w_dma(ch):
                    wt = wp.tile([128, CHUNK_G, 4, JD], FP8 if W_FP8 else BF16, tag="wt")
                    # alternate the two HWDGE rings so two chunks are always in
                    # flight; all PSUM->SBUF casts run on Vector so the scalar
                    # ring's W issues are never queued behind a copy
                    eng = nc.sync if ch % 2 == 0 else nc.scalar
                    eng.dma_start(wt[:], wt_in[ch])
                    return wt

                wts = [w_dma(0), w_dma(1), w_dma(2), w_dma(3)]
                nsub = 0
                for ch in range(NCHUNK):
                    wt = wts.pop(0)
                    if ch + 4 < NCHUNK:
                        wts.append(w_dma(ch + 4))
                    for gg in range(CHUNK_G):
                        g = ch * CHUNK_G + gg
                        pg = ps1.tile([128, 2, JD], F32, tag="pg")
                        for rp in range(2):
                            for c in range(4):
                                nc.tensor.matmul(
                                    pg[32 * c : 32 * c + 32, rp, :],
                                    xs[64 * rp : 64 * rp + 64, g, c, :],
                                    wt[64 * rp : 64 * rp + 64, gg, c, :],
                                    tile_position=(64 * rp, 32 * c),
                                )
                        dst = C[:, g].rearrange("p rp d j -> p rp (d j)")
                        # tail copies go to ACT: by then every W chunk has
                        # been issued, so the scalar ring has nothing to delay
                        if g >= NG - 6 and g % 2 == 0:
                            nc.scalar.copy(dst, pg[:])
                        else:
                            nc.vector.tensor_copy(dst, pg[:])
                        nsub += 1
                        if nsub > 2:
                            g2 = nsub - 3
                            s0_slice(g2, 0)
                            s0_slice(g2, 1)
                for g2 in range(NG - 2, NG):
                    s0_slice(g2, 0)
                    s0_slice(g2, 1)

            # ---- routing ----
            with tc.tile_pool(name="blk", bufs=2) as bp:
                for it in range(ROUTINGS):
                    # squash factor from s_ps (all on [B_LOC(=4), ...]);
                    # Square first: the n2 chain is the critical path, the
                    # s_gb copy only feeds the later replication matmul
                    sq = sp.tile([B_LOC, D, J], F32, tag="sq")
                    nc.scalar.activation(
                        sq.rearrange("b d j -> b (d j)"), s_ps[:], Act.Square
                    )
                    s_gb = sp.tile([B_LOC, D, J], BF16, tag="s_gb")
                    nc.scalar.copy(s_gb.rearrange("b d j -> b (d j)"), s_ps[:])
                    n2 = sp.tile([B_LOC, J], F32, tag="n2")
                    nc.vector.tensor_reduce(
                        n2[:],
                        sq.rearrange("b d j -> b j d"),
                        axis=mybir.AxisListType.X,
                        op=Alu.add,
                    )
                    # factor = n2 / (1 + n2) / sqrt(n2 + eps); rsqrt via a
                    # 2-step Newton iteration with the quake bitcast seed —
                    # all DVE, so the ACT engine never leaves the exp table
                    # set (a Sqrt/Ln activation would reload tables ~2.6us
                    # per iteration)
                    n2e = sp.tile([B_LOC, J], F32, tag="n2e")
                    nc.vector.tensor_scalar_add(n2e[:], n2[:], EPS)
                    hf = sp.tile([B_LOC, J], F32, tag="hf")
                    nc.vector.tensor_scalar_mul(hf[:], n2e[:], 0.5)
                    zi = sp.tile([B_LOC, J], U32, tag="zi")
                    nc.vector.tensor_tensor(
                        zi[:], n2e.bitcast(U32),
                        rc_one.broadcast_to([B_LOC, J]),
                        op=Alu.logical_shift_right,
                    )
                    ry = sp.tile([B_LOC, J], F32, tag="ry")
                    nc.vector.tensor_tensor(
                        ry.bitcast(U32), rc_magic.broadcast_to([B_LOC, J]),
                        zi[:], op=Alu.subtract,
                    )
                    rt = sp.tile([B_LOC, J], F32, tag="rt")
                    for _ in range(2):
                        nc.vector.tensor_tensor(rt[:], ry[:], ry[:], op=Alu.mult)
                        nc.vector.tensor_tensor(rt[:], rt[:], hf[:], op=Alu.mult)
                        nc.vector.tensor_tensor(
                            rt[:], rc_15.broadcast_to([B_LOC, J]), rt[:],
                            op=Alu.subtract,
                        )
                        nc.vector.tensor_tensor(ry[:], ry[:], rt[:], op=Alu.mult)
                    tmp = sp.tile([B_LOC, J], F32, tag="tmp")
                    nc.vector.tensor_scalar_add(tmp[:], n2[:], 1.0)
                    qr = sp.tile([B_LOC, J], F32, tag="qr")
                    nc.vector.reciprocal(qr[:], tmp[:])
                    fac = sp.tile([B_LOC, J], F32, tag="fac")
                    nc.vector.tensor_tensor(fac[:], qr[:], ry[:], op=Alu.mult)

                    if it < ROUTINGS - 1:
                        facb = sp.tile([B_LOC, J], BF16, tag="facb")
                        nc.vector.tensor_tensor(facb[:], fac[:], n2[:], op=Alu.mult)
                        # replicate s and fac over the 32 strips via the PE
                        nc.tensor.matmul(
                            vr_ps[:], selT, s_gb.rearrange("b d j -> b (d j)")
                        )
                        nc.tensor.matmul(fr_ps[:], selT, facb[:])
                        fr_sb = sp.tile([128, J], BF16, tag="fr_sb")
                        nc.scalar.copy(fr_sb[:], fr_ps[:])
                        nc.vector.tensor_tensor(
                            v_rep[:],
                            vr_ps.rearrange("p (d j) -> p d j", d=D, j=J),
                            fr_sb[:, None, :].broadcast_to([128, D, J]),
                            op=Alu.mult,
                        )
                        # fused per-block routing step. The j-softmax is
                        # slice-local, so each block runs its ENTIRE chain —
                        # agreement (log2 tree over d), logit update, exp,
                        # block-local softmax sum (folded into the s-matmul
                        # selector), coefficient product, s-matmuls — before
                        # the next block. The s-matmul chain (13.8 us serial
                        # on the PE) then overlaps the remaining blocks' DVE
                        # work instead of all queueing after pass A.
                        for blk in range(NBLK):
                            gs = slice(blk * GB, blk * GB + GB)
                            Cb = C[:, gs]
                            pi2 = bp.tile([128, GB, 2, D, J], BF16, tag="pi2")
                            nc.vector.tensor_tensor(
                                pi2[:],
                                Cb,
                                v_rep[:, None, None, :, :].broadcast_to(
                                    [128, GB, 2, D, J]
                                ),
                                op=Alu.mult,
                            )
                            t8 = bp.tile([128, GB, 2, 8, J], BF16, tag="t8")
                            nc.vector.tensor_tensor(
                                t8[:], pi2[:, :, :, 0:8, :], pi2[:, :, :, 8:16, :],
                                op=Alu.add,
                            )
                            t4 = bp.tile([128, GB, 2, 4, J], BF16, tag="t4")
                            nc.vector.tensor_tensor(
                                t4[:], t8[:, :, :, 0:4, :], t8[:, :, :, 4:8, :],
                                op=Alu.add,
                            )
                            t2 = bp.tile([128, GB, 2, 2, J], BF16, tag="t2")
                            nc.vector.tensor_tensor(
                                t2[:], t4[:, :, :, 0:2, :], t4[:, :, :, 2:4, :],
                                op=Alu.add,
                            )
                            t1 = bp.tile([128, GB, 2, J], BF16, tag="t1")
                            nc.vector.tensor_tensor(
                                t1[:], t2[:, :, :, 0, :], t2[:, :, :, 1, :],
                                op=Alu.add,
                            )
                            if it == 0:
                                nc.vector.tensor_copy(bl[:, gs], t1[:])
                            else:
                                nc.vector.tensor_add(bl[:, gs], bl[:, gs], t1[:])
                            nc.scalar.activation(p_t[:, gs], bl[:, gs], Act.Exp)
                            S = bp.tile([128, GB, 2], F32, tag="S")
                            nc.vector.tensor_reduce(
                                S[:], p_t[:, gs], axis=mybir.AxisListType.X,
                                op=Alu.add,
                            )
                            Sr = bp.tile([128, GB, 2], BF16, tag="Sr")
                            with nc.allow_low_precision(
                                reason="softmax divisor in bf16: c is consumed "
                                "in bf16 products anyway"
                            ):
                                nc.vector.reciprocal(Sr[:], S[:])
                            sel_s = bp.tile([128, GB, 2, B_LOC], BF16, tag="sel_s")
                            nc.vector.tensor_tensor(
                                sel_s[:],
                                sel[:, None, None, :].broadcast_to(
                                    [128, GB, 2, B_LOC]
                                ),
                                Sr[:, :, :, None].broadcast_to(
                                    [128, GB, 2, B_LOC]
                                ),
                                op=Alu.mult,
                            )
                            nsplit = 4 if blk == NBLK - 1 else 1
                            for sb in range(nsplit):
                                gw = GB // nsplit
                                goff = blk * GB + sb * gw
                                gsp = slice(goff, goff + gw)
                                pi = bp.tile([128, GB, 2, D, J], BF16, tag="pi")
                                piv = pi[:, 0:gw]
                                nc.vector.tensor_tensor(
                                    piv,
                                    C[:, gsp],
                                    p_t[:, gsp, :, None, :].broadcast_to(
                                        [128, gw, 2, D, J]
                                    ),
                                    op=Alu.mult,
                                )
                                for gr in range(gw * 2):
                                    g2, rp = divmod(gr, 2)
                                    kk = goff * 2 + gr
                                    nc.tensor.matmul(
                                        s_ps[:],
                                        sel_s[:, sb * gw + g2, rp, :],
                                        piv[:, g2, rp].rearrange(
                                            "p d j -> p (d j)"
                                        ),
                                        start=(kk == 0),
                                        stop=(kk == 2 * NG - 1),
                                    )
                    else:
                        # final output: v = s * fac in f32, (d, j) -> (j, d)
                        facf = sp.tile([B_LOC, J], F32, tag="facf")
                        nc.vector.tensor_tensor(
                            facf[:], fac[:], n2[:], op=Alu.mult
                        )
                        v_jd = sp.tile([B_LOC, J, D], F32, tag="v_jd")
                        nc.vector.tensor_tensor(
                            v_jd[:],
                            s_gb.rearrange("b d j -> b j d"),
                            facf[:, :, None].broadcast_to([B_LOC, J, D]),
                            op=Alu.mult,
                        )
                        nc.sync.dma_start(v_out[:], v_jd[:])

    nc.compile()
    return nc


def _prep_inputs(x, W):
    """Host-side layout prep (bf16). W is shared by all cores; x is B-sliced."""
    # wt[ch, p=(rp,i8,k), gg, c, (d j)] = W[j, i, d, k],
    # i = ((g*2+rp)*4+c)*8 + i8, g = ch*CHUNK_G + gg
    Wr = W.reshape(J, NG, 2, 4, 8, D, K)                  # j g rp c i8 d k
    wt = np.ascontiguousarray(Wr.transpose(1, 2, 4, 6, 3, 5, 0))  # g rp i8 k c d j
    wt = wt.reshape(NCHUNK, CHUNK_G, 128, 4, JD).transpose(0, 2, 1, 3, 4)
    if W_FP8:
        wt = np.ascontiguousarray(wt * W_SCALE).astype(ml_dtypes.float8_e4m3)
    else:
        wt = np.ascontiguousarray(wt).astype(ml_dtypes.bfloat16)
    i8 = np.arange(8)
    if W_FP8:
        x = x / W_SCALE  # exact in bf16 (power-of-2); undoes the W scale
    in_maps = []
    for m in range(NCORES):
        xb = x[B_LOC * m : B_LOC * (m + 1)].reshape(B_LOC, NG, 2, 4, 8, K)
        xsz = np.zeros((2, 8, K, NG, 4, 8, B_LOC), np.float32)  # rp i8 k g c i8' b
        xsz[:, i8, :, :, :, i8, :] = xb.transpose(4, 2, 5, 1, 3, 0)[i8]
        in_maps.append(
            {
                "wt": wt,
                "xs": xsz.reshape(128, NG, 4, 32).astype(ml_dtypes.bfloat16),
            }
        )
    return in_maps


def run(inputs, trace=False):
    if "nc" not in _CACHE:
        _CACHE["nc"] = _build()
    nc = _CACHE["nc"]
    in_maps = _prep_inputs(np.asarray(inputs["x"]), np.asarray(inputs["W"]))
    bkr = run_bass_kernel_spmd(
        nc, in_maps, core_ids=list(range(NCORES)), trace=trace
    )
    out = np.concatenate(
        [bkr.results[m]["v"].astype(np.float32) for m in range(NCORES)], axis=0
    )
    return out, bkr


def kernel(x, W):
    out, _ = run({"x": np.asarray(x), "W": np.asarray(W)})
    return out


# revision 50
# speedup vs baseline: 1.1967x; 1.0036x over previous
"""Trainium2 Bass kernel for the Capsule routing layer (nn_Capsule_49658411876931).

Math (see reference):
    u_hat[b,j,i,d] = sum_k W[j,i,d,k] * x[b,i,k]
    b0 = 0
    for r in 0..2:
        c = softmax(b, axis=j)
        s[b,j,d] = sum_i c[b,j,i] u_hat[b,j,i,d]
        v = squash(s)  (over d)
        if r < 2: b += sum_d u_hat[b,j,i,d] v[b,j,d]
    return v  [B, J, D]

Sharding: batch B=32 split over 8 cores (B_LOC=4); W replicated (16.8 MB
bf16 per core, streamed once from HBM in 1 MB chunks at ~340 GB/s). The
routing loop is then fully core-local: no collectives at all (the previous
I-sharded design lost ~90 us to three ncfw AllReduce latencies).

Per-core layouts (P = SBUF partition index):
  i = ((g*2 + rp)*4 + c)*8 + i8   (g in 0..31, rp in 0..1, c in 0..3, i8 in 0..7)
  u_hat "C" tensor : [P = 32c + 4*i8 + b, free = (g, rp, d, j)]  bf16
  logits / c       : [P = 32c + 4*i8 + b, free = (g, rp, j)]
u_hat is computed with 8-way tile_position-packed PE matmuls: stationary is a
host-prepped block-diagonal x chunk [64 rows = (i8,k), 32 cols = (i8',b)]
(zero off-diagonal), moving is the W chunk [64, 512]; each matmul covers 8
input capsules, 8 matmuls (rp, c) run concurrently per round g.

s[b,dj] = sum_i c*u_hat runs on the PE as 64 accumulating matmuls against a
"collapse the 32 (c,i8)-strips" selector stationary (delta_{p%4,b}); for
iteration 0 the softmax coefficients are the constant 1/J, folded into a
(1/J)-scaled selector so no elementwise pass is needed. In iterations 1-2
the 1/sum_j exp softmax divisor is folded into the selector as well
(per-partition-and-slice scale), skipping the c = p_t/S elementwise pass.
The agreement d-contraction runs as a log2 tree of bf16 2x-mode
tensor_tensor adds. The j-softmax is slice-local, so each routing block
runs its full chain (agreement tree -> logits -> exp -> block softmax ->
coefficient product -> s-matmuls) before the next block: the 13.8 us
serial s-matmul chain on the PE overlaps the remaining blocks' DVE work.
squash's rsqrt is a 2-step Newton iteration on the DVE (quake bitcast
seed), so the ACT engine never leaves the exp table set (a Sqrt/Ln
activation would reload tables ~2.6 us per iteration; measured).

Rejected with evidence: fp8e4 W (rel err 2.47e-2 > the 2e-2 gate, scale-
invariant - pure 3-bit-mantissa error), GpSimd elementwise offload (~6.5us
per 512-el/partition tensor_tensor vs 0.6us on DVE - dispatch dominated),
single-ring or SWDGE W streaming (275 GB/s vs 323 on alternating HWDGE
rings), 2 MB W chunks with prefetch depth 1 (+9 us).
"""

import numpy as np
import ml_dtypes

import concourse.bass as bass
import concourse.tile as tile
from concourse import bacc, mybir
from concourse.bass_utils import run_bass_kernel_spmd

F32 = mybir.dt.float32
BF16 = mybir.dt.bfloat16
FP8 = mybir.dt.float8e4
Alu = mybir.AluOpType
Act = mybir.ActivationFunctionType

B, I, K = 32, 2048, 8
J, D = 32, 16
JD = J * D                     # 512
NCORES = 8
B_LOC = B // NCORES            # 4
NG = 32                        # rounds g; i = ((g*2+rp)*4+c)*8 + i8
CHUNK_G = 2                    # g's per W DMA chunk (1 MB bf16)
NCHUNK = NG // CHUNK_G         # 16
W_FP8 = False                  # fp8 W fails the 2e-2 gate (measured 2.5e-2)
W_SCALE = 64.0                 # power-of-2, folded exactly into bf16 xs
GB = 4                         # g's per routing block
NBLK = NG // GB                # 8 routing blocks (8 (g,rp)-slices each)
ROUTINGS = 3
EPS = 1e-7

_CACHE = {}


def _build():
    nc = bacc.Bacc("TRN2", target_bir_lowering=False, debug=False, num_devices=NCORES)

    wt_in = nc.dram_tensor(
        "wt", [NCHUNK, 128, CHUNK_G, 4, JD], FP8 if W_FP8 else BF16,
        kind="ExternalInput",
    )
    xs_in = nc.dram_tensor("xs", [128, NG, 4, 32], BF16, kind="ExternalInput")
    v_out = nc.dram_tensor("v", [B_LOC, J, D], F32, kind="ExternalOutput")

    # Selector constants for cross-partition PE ops:
    #   sel[p, b'] = 1 iff p % 4 == b'     (collapse the 32 (c,i8)-strips)
    #   sel32 = sel / J                    (fold in the uniform iter-0 softmax)
    #   selT[b, p] = sel^T                 (replicate over the 32 strips)
    p_idx = np.arange(128)
    sel_np = (p_idx[:, None] % B_LOC == np.arange(B_LOC)[None, :]).astype(np.float32)
    selpack = np.zeros((128, 2 * B_LOC + 128), np.float32)
    selpack[:, 0:B_LOC] = sel_np
    selpack[:, B_LOC : 2 * B_LOC] = sel_np / J
    selpack[0:B_LOC, 2 * B_LOC :] = sel_np.T
    sel_dram = nc.inline_tensor(selpack.astype(ml_dtypes.bfloat16), "selpack")

    with tile.TileContext(nc) as tc:
        with (
            tc.tile_pool(name="persist", bufs=1) as pp,
            tc.tile_pool(name="small", bufs=1) as sp,
            tc.tile_pool(name="spsum", bufs=1, space="PSUM") as ssp,
        ):
            # ---- persistent SBUF tensors ----
            xs = pp.tile([128, NG, 4, 32], BF16)        # block-diag x stationaries
            C = pp.tile([128, NG, 2, D, J], BF16)       # u_hat
            bl = pp.tile([128, NG, 2, J], F32)          # routing logits
            p_t = pp.tile([128, NG, 2, J], BF16)        # exp(b)
            selc = pp.tile([128, 2 * B_LOC + 128], BF16)
            v_rep = pp.tile([128, D, J], BF16)          # v replicated over strips

            sel = selc[:, 0:B_LOC]
            sel32 = selc[:, B_LOC : 2 * B_LOC]
            selT = selc[0:B_LOC, 2 * B_LOC :]

            # persistent PSUM: s accumulator + v/fac replication banks
            s_ps = ssp.tile([B_LOC, D * J], F32)
            vr_ps = ssp.tile([128, D * J], F32)
            fr_ps = ssp.tile([128, J], F32)

            # a tiny leading xs slice (g 0-1, 64 KB) unblocks the first
            # matmuls ~3us earlier than a monolithic xs transfer; the rest
            # streams behind it. selc trails (only needed once s0 starts).
            # No bl memset: iteration 0's logit update writes bl.
            for qs in (slice(0, 2), slice(2, 8), slice(8, 16),
                       slice(16, 24), slice(24, 32)):
                nc.gpsimd.dma_start(xs[:, qs], xs_in[:, qs])
            nc.scalar.dma_start(selc[:], sel_dram[:])

            # Warm the ACT exp table set at t~0 (under the W DMA shadow) so
            # the first softmax doesn't stall ~2.7us. The squash sqrt runs as
            # a DVE Newton rsqrt, so exp is the only table set ever loaded.
            wa = sp.tile([1, 8], F32, tag="wa")
            wb = sp.tile([1, 8], F32, tag="wb")
            nc.vector.memset(wa[:], 1.0)
            nc.scalar.activation(wb[:], wa[:], Act.Exp)

            # constants for the Newton rsqrt (quake seed)
            U32 = mybir.dt.uint32
            rc_magic = sp.tile([B_LOC, 1], U32, tag="rc_magic")
            rc_one = sp.tile([B_LOC, 1], U32, tag="rc_one")
            rc_15 = sp.tile([B_LOC, 1], F32, tag="rc_15")
            nc.vector.memset(rc_magic[:], 0x5F3759DF)
            nc.vector.memset(rc_one[:], 1)
            nc.vector.memset(rc_15[:], 1.5)

            # ---- phase 1: u_hat + iteration-0 s accumulation ----
            ns0 = [0]

            def s0_slice(g, rp):
                kk = ns0[0]
                ns0[0] += 1
                nc.tensor.matmul(
                    s_ps[:],
                    sel32,
                    C[:, g, rp].rearrange("p d j -> p (d j)"),
                    start=(kk == 0),
                    stop=(kk == 2 * NG - 1),
                )

            with (
                tc.tile_pool(name="wpool", bufs=5) as wp,
                tc.tile_pool(name="psum1", bufs=2, space="PSUM") as ps1,
            ):
                def w_dma(ch):
                    wt = wp.tile([128, CHUNK_G, 4, JD], FP8 if W_FP8 else BF16, tag="wt")
                    # alternate the two HWDGE rings so two chunks are always
                    # in flight; the head chunks are split per-g (each ring is
                    # FIFO, so a smaller first transfer completes sooner and
                    # un-gates the first matmuls/casts earlier)
                    eng = nc.sync if ch % 2 == 0 else nc.scalar
                    if ch < 2:
                        for gg in range(CHUNK_G):
                            eng.dma_start(
                                wt[:, gg : gg + 1], wt_in[ch, :, gg : gg + 1]
                            )
                    else:
                        eng.dma_start(wt[:], wt_in[ch])
                    return wt

                wts = [w_dma(0), w_dma(1), w_dma(2), w_dma(3)]
                nsub = 0
                for ch in range(NCHUNK):
                    wt = wts.pop(0)
                    if ch + 4 < NCHUNK:
                        wts.append(w_dma(ch + 4))
                    for gg in range(CHUNK_G):
                        g = ch * CHUNK_G + gg
                        pg = ps1.tile([128, 2, JD], F32, tag="pg")
                        for rp in range(2):
                            for c in range(4):
                                nc.tensor.matmul(
                                    pg[32 * c : 32 * c + 32, rp, :],
                                    xs[64 * rp : 64 * rp + 64, g, c, :],
                                    wt[64 * rp : 64 * rp + 64, gg, c, :],
                                    tile_position=(64 * rp, 32 * c),
                                )
                        dst = C[:, g].rearrange("p rp d j -> p rp (d j)")
                        # alternate casts DVE/ACT: doubles copy-chain
                        # throughput (the phase-1 tail binds on it); the
                        # 4-deep W prefetch absorbs the <=1us issue delay a
                        # copy can add ahead of a scalar-ring W DIRECT2D
                        if g % 2 == 1:
                            nc.scalar.copy(dst, pg[:])
                        else:
                            nc.vector.tensor_copy(dst, pg[:])
                        nsub += 1
                        if nsub > 2:
                            g2 = nsub - 3
                            s0_slice(g2, 0)
                            s0_slice(g2, 1)
                for g2 in range(NG - 2, NG):
                    s0_slice(g2, 0)
                    s0_slice(g2, 1)

            # ---- routing ----
            with tc.tile_pool(name="blk", bufs=2) as bp:
                for it in range(ROUTINGS):
                    # squash factor from s_ps (all on [B_LOC(=4), ...]);
                    # Square first: the n2 chain is the critical path, the
                    # s_gb copy only feeds the later replication matmul
                    sq = sp.tile([B_LOC, D, J], F32, tag="sq")
                    nc.scalar.activation(
                        sq.rearrange("b d j -> b (d j)"), s_ps[:], Act.Square
                    )
                    s_gb = sp.tile([B_LOC, D, J], BF16, tag="s_gb")
                    nc.scalar.copy(s_gb.rearrange("b d j -> b (d j)"), s_ps[:])
                    n2 = sp.tile([B_LOC, J], F32, tag="n2")
                    nc.vector.tensor_reduce(
                        n2[:],
                        sq.rearrange("b d j -> b j d"),
                        axis=mybir.AxisListType.X,
                        op=Alu.add,
                    )
                    # factor = n2 / (1 + n2) / sqrt(n2 + eps); rsqrt via a
                    # 2-step Newton iteration with the quake bitcast seed —
                    # all DVE, so the ACT engine never leaves the exp table
                    # set (a Sqrt/Ln activation would reload tables ~2.6us
                    # per iteration)
                    n2e = sp.tile([B_LOC, J], F32, tag="n2e")
                    nc.vector.tensor_scalar_add(n2e[:], n2[:], EPS)
                    hf = sp.tile([B_LOC, J], F32, tag="hf")
                    nc.vector.tensor_scalar_mul(hf[:], n2e[:], 0.5)
                    zi = sp.tile([B_LOC, J], U32, tag="zi")
                    nc.vector.tensor_tensor(
                        zi[:], n2e.bitcast(U32),
                        rc_one.broadcast_to([B_LOC, J]),
                        op=Alu.logical_shift_right,
                    )
                    ry = sp.tile([B_LOC, J], F32, tag="ry")
                    nc.vector.tensor_tensor(
                        ry.bitcast(U32), rc_magic.broadcast_to([B_LOC, J]),
                        zi[:], op=Alu.subtract,
                    )
                    rt = sp.tile([B_LOC, J], F32, tag="rt")
                    for _ in range(2):
                        nc.vector.tensor_tensor(rt[:], ry[:], ry[:], op=Alu.mult)
                        nc.vector.tensor_tensor(rt[:], rt[:], hf[:], op=Alu.mult)
                        nc.vector.tensor_tensor(
                            rt[:], rc_15.broadcast_to([B_LOC, J]), rt[:],
                            op=Alu.subtract,
                        )
                        nc.vector.tensor_tensor(ry[:], ry[:], rt[:], op=Alu.mult)
                    tmp = sp.tile([B_LOC, J], F32, tag="tmp")
                    nc.vector.tensor_scalar_add(tmp[:], n2[:], 1.0)
                    qr = sp.tile([B_LOC, J], F32, tag="qr")
                    nc.vector.reciprocal(qr[:], tmp[:])
                    fac = sp.tile([B_LOC, J], F32, tag="fac")
                    nc.vector.tensor_tensor(fac[:], qr[:], ry[:], op=Alu.mult)

                    if it < ROUTINGS - 1:
                        facb = sp.tile([B_LOC, J], BF16, tag="facb")
                        nc.vector.tensor_tensor(facb[:], fac[:], n2[:], op=Alu.mult)
                        # replicate s and fac over the 32 strips via the PE
                        nc.tensor.matmul(
                            vr_ps[:], selT, s_gb.rearrange("b d j -> b (d j)")
                        )
                        nc.tensor.matmul(fr_ps[:], selT, facb[:])
                        fr_sb = sp.tile([128, J], BF16, tag="fr_sb")
                        nc.scalar.copy(fr_sb[:], fr_ps[:])
                        nc.vector.tensor_tensor(
                            v_rep[:],
                            vr_ps.rearrange("p (d j) -> p d j", d=D, j=J),
                            fr_sb[:, None, :].broadcast_to([128, D, J]),
                            op=Alu.mult,
                        )
                        # fused per-block routing step. The j-softmax is
                        # slice-local, so each block runs its ENTIRE chain —
                        # agreement (log2 tree over d), logit update, exp,
                        # block-local softmax sum (folded into the s-matmul
                        # selector), coefficient product, s-matmuls — before
                        # the next block. The s-matmul chain (13.8 us serial
                        # on the PE) then overlaps the remaining blocks' DVE
                        # work instead of all queueing after pass A.
                        for blk in range(NBLK):
                            gs = slice(blk * GB, blk * GB + GB)
                            Cb = C[:, gs]
                            pi2 = bp.tile([128, GB, 2, D, J], BF16, tag="pi2")
                            nc.vector.tensor_tensor(
                                pi2[:],
                                Cb,
                                v_rep[:, None, None, :, :].broadcast_to(
                                    [128, GB, 2, D, J]
                                ),
                                op=Alu.mult,
                            )
                            t8 = bp.tile([128, GB, 2, 8, J], BF16, tag="t8")
                            nc.vector.tensor_tensor(
                                t8[:], pi2[:, :, :, 0:8, :], pi2[:, :, :, 8:16, :],
                                op=Alu.add,
                            )
                            t4 = bp.tile([128, GB, 2, 4, J], BF16, tag="t4")
                            nc.vector.tensor_tensor(
                                t4[:], t8[:, :, :, 0:4, :], t8[:, :, :, 4:8, :],
                                op=Alu.add,
                            )
                            t2 = bp.tile([128, GB, 2, 2, J], BF16, tag="t2")
                            nc.vector.tensor_tensor(
                                t2[:], t4[:, :, :, 0:2, :], t4[:, :, :, 2:4, :],
                                op=Alu.add,
                            )
                            t1 = bp.tile([128, GB, 2, J], BF16, tag="t1")
                            nc.vector.tensor_tensor(
                                t1[:], t2[:, :, :, 0, :], t2[:, :, :, 1, :],
                                op=Alu.add,
                            )
                            if it == 0:
                                nc.vector.tensor_copy(bl[:, gs], t1[:])
                            else:
                                nc.vector.tensor_add(bl[:, gs], bl[:, gs], t1[:])
                            nc.scalar.activation(p_t[:, gs], bl[:, gs], Act.Exp)
                            S = bp.tile([128, GB, 2], F32, tag="S")
                            nc.vector.tensor_reduce(
                                S[:], p_t[:, gs], axis=mybir.AxisListType.X,
                                op=Alu.add,
                            )
                            Sr = bp.tile([128, GB, 2], BF16, tag="Sr")
                            with nc.allow_low_precision(
                                reason="softmax divisor in bf16: c is consumed "
                                "in bf16 products anyway"
                            ):
                                nc.vector.reciprocal(Sr[:], S[:])
                            sel_s = bp.tile([128, GB, 2, B_LOC], BF16, tag="sel_s")
                            nc.vector.tensor_tensor(
                                sel_s[:],
                                sel[:, None, None, :].broadcast_to(
                                    [128, GB, 2, B_LOC]
                                ),
                                Sr[:, :, :, None].broadcast_to(
                                    [128, GB, 2, B_LOC]
                                ),
                                op=Alu.mult,
                            )
                            nsplit = 4 if blk == NBLK - 1 else 1
                            for sb in range(nsplit):
                                gw = GB // nsplit
                                goff = blk * GB + sb * gw
                                gsp = slice(goff, goff + gw)
                                pi = bp.tile([128, GB, 2, D, J], BF16, tag="pi")
                                piv = pi[:, 0:gw]
                                nc.vector.tensor_tensor(
                                    piv,
                                    C[:, gsp],
                                    p_t[:, gsp, :, None, :].broadcast_to(
                                        [128, gw, 2, D, J]
                                    ),
                                    op=Alu.mult,
                                )
                                for gr in range(gw * 2):
                                    g2, rp = divmod(gr, 2)
                                    kk = goff * 2 + gr
                                    nc.tensor.matmul(
                                        s_ps[:],
                                        sel_s[:, sb * gw + g2, rp, :],
                                        piv[:, g2, rp].rearrange(
                                            "p d j -> p (d j)"
                                        ),
                                        start=(kk == 0),
                                        stop=(kk == 2 * NG - 1),
                                    )
                    else:
                        # final output: v = s * fac in f32, (d, j) -> (j, d)
                        facf = sp.tile([B_LOC, J], F32, tag="facf")
                        nc.vector.tensor_tensor(
                            facf[:], fac[:], n2[:], op=Alu.mult
                        )
                        v_jd = sp.tile([B_LOC, J, D], F32, tag="v_jd")
                        nc.vector.tensor_tensor(
                            v_jd[:],
                            s_gb.rearrange("b d j -> b j d"),
                            facf[:, :, None].broadcast_to([B_LOC, J, D]),
                            op=Alu.mult,
                        )
                        nc.sync.dma_start(v_out[:], v_jd[:])

    nc.compile()
    return nc


def _prep_inputs(x, W):
    """Host-side layout prep (bf16). W is shared by all cores; x is B-sliced."""
    # wt[ch, p=(rp,i8,k), gg, c, (d j)] = W[j, i, d, k],
    # i = ((g*2+rp)*4+c)*8 + i8, g = ch*CHUNK_G + gg
    Wr = W.reshape(J, NG, 2, 4, 8, D, K)                  # j g rp c i8 d k
    wt = np.ascontiguousarray(Wr.transpose(1, 2, 4, 6, 3, 5, 0))  # g rp i8 k c d j
    wt = wt.reshape(NCHUNK, CHUNK_G, 128, 4, JD).transpose(0, 2, 1, 3, 4)
    if W_FP8:
        wt = np.ascontiguousarray(wt * W_SCALE).astype(ml_dtypes.float8_e4m3)
    else:
        wt = np.ascontiguousarray(wt).astype(ml_dtypes.bfloat16)
    i8 = np.arange(8)
    if W_FP8:
        x = x / W_SCALE  # exact in bf16 (power-of-2); undoes the W scale
    in_maps = []
    for m in range(NCORES):
        xb = x[B_LOC * m : B_LOC * (m + 1)].reshape(B_LOC, NG, 2, 4, 8, K)
        xsz = np.zeros((2, 8, K, NG, 4, 8, B_LOC), np.float32)  # rp i8 k g c i8' b
        xsz[:, i8, :, :, :, i8, :] = xb.transpose(4, 2, 5, 1, 3, 0)[i8]
        in_maps.append(
            {
                "wt": wt,
                "xs": xsz.reshape(128, NG, 4, 32).astype(ml_dtypes.bfloat16),
            }
        )
    return in_maps


def run(inputs, trace=False):
    if "nc" not in _CACHE:
        _CACHE["nc"] = _build()
    nc = _CACHE["nc"]
    in_maps = _prep_inputs(np.asarray(inputs["x"]), np.asarray(inputs["W"]))
    bkr = run_bass_kernel_spmd(
        nc, in_maps, core_ids=list(range(NCORES)), trace=trace
    )
    out = np.concatenate(
        [bkr.results[m]["v"].astype(np.float32) for m in range(NCORES)], axis=0
    )
    return out, bkr


def kernel(x, W):
    out, _ = run({"x": np.asarray(x), "W": np.asarray(W)})
    return out


# revision 51
# speedup vs baseline: 1.2041x; 1.0062x over previous
"""Trainium2 Bass kernel for the Capsule routing layer (nn_Capsule_49658411876931).

Math (see reference):
    u_hat[b,j,i,d] = sum_k W[j,i,d,k] * x[b,i,k]
    b0 = 0
    for r in 0..2:
        c = softmax(b, axis=j)
        s[b,j,d] = sum_i c[b,j,i] u_hat[b,j,i,d]
        v = squash(s)  (over d)
        if r < 2: b += sum_d u_hat[b,j,i,d] v[b,j,d]
    return v  [B, J, D]

Sharding: batch B=32 split over 8 cores (B_LOC=4); W replicated (16.8 MB
bf16 per core, streamed once from HBM in 1 MB chunks at ~340 GB/s). The
routing loop is then fully core-local: no collectives at all (the previous
I-sharded design lost ~90 us to three ncfw AllReduce latencies).

Per-core layouts (P = SBUF partition index):
  i = ((g*2 + rp)*4 + c)*8 + i8   (g in 0..31, rp in 0..1, c in 0..3, i8 in 0..7)
  u_hat "C" tensor : [P = 32c + 4*i8 + b, free = (g, rp, d, j)]  bf16
  logits / c       : [P = 32c + 4*i8 + b, free = (g, rp, j)]
u_hat is computed with 8-way tile_position-packed PE matmuls: stationary is a
host-prepped block-diagonal x chunk [64 rows = (i8,k), 32 cols = (i8',b)]
(zero off-diagonal), moving is the W chunk [64, 512]; each matmul covers 8
input capsules, 8 matmuls (rp, c) run concurrently per round g.

s[b,dj] = sum_i c*u_hat runs on the PE as 64 accumulating matmuls against a
"collapse the 32 (c,i8)-strips" selector stationary (delta_{p%4,b}); for
iteration 0 the softmax coefficients are the constant 1/J, folded into a
(1/J)-scaled selector so no elementwise pass is needed. In iterations 1-2
the 1/sum_j exp softmax divisor is folded into the selector as well
(per-partition-and-slice scale), skipping the c = p_t/S elementwise pass.
The agreement d-contraction runs as a log2 tree of bf16 2x-mode
tensor_tensor adds. The j-softmax is slice-local, so each routing block
runs its full chain (agreement tree -> logits -> exp -> block softmax ->
coefficient product -> s-matmuls) before the next block: the 13.8 us
serial s-matmul chain on the PE overlaps the remaining blocks' DVE work.
squash's rsqrt is a 2-step Newton iteration on the DVE (quake bitcast
seed), so the ACT engine never leaves the exp table set (a Sqrt/Ln
activation would reload tables ~2.6 us per iteration; measured).

Rejected with evidence: fp8e4 W (rel err 2.47e-2 > the 2e-2 gate, scale-
invariant - pure 3-bit-mantissa error), GpSimd elementwise offload (~6.5us
per 512-el/partition tensor_tensor vs 0.6us on DVE - dispatch dominated),
single-ring or SWDGE W streaming (275 GB/s vs 323 on alternating HWDGE
rings), 2 MB W chunks with prefetch depth 1 (+9 us).
"""

import numpy as np
import ml_dtypes

import concourse.bass as bass
import concourse.tile as tile
from concourse import bacc, mybir
from concourse.bass_utils import run_bass_kernel_spmd

F32 = mybir.dt.float32
BF16 = mybir.dt.bfloat16
FP8 = mybir.dt.float8e4
Alu = mybir.AluOpType
Act = mybir.ActivationFunctionType

B, I, K = 32, 2048, 8
J, D = 32, 16
JD = J * D                     # 512
NCORES = 8
B_LOC = B // NCORES            # 4
NG = 32                        # rounds g; i = ((g*2+rp)*4+c)*8 + i8
CHUNK_G = 2                    # g's per W DMA chunk (1 MB bf16)
NCHUNK = NG // CHUNK_G         # 16
W_FP8 = False                  # fp8 W fails the 2e-2 gate (measured 2.5e-2)
W_SCALE = 64.0                 # power-of-2, folded exactly into bf16 xs
GB = 4                         # g's per routing block
NBLK = NG // GB                # 8 routing blocks (8 (g,rp)-slices each)
ROUTINGS = 3
EPS = 1e-7

_CACHE = {}


def _build():
    nc = bacc.Bacc("TRN2", target_bir_lowering=False, debug=False, num_devices=NCORES)

    wt_in = nc.dram_tensor(
        "wt", [NCHUNK, 128, CHUNK_G, 4, JD], FP8 if W_FP8 else BF16,
        kind="ExternalInput",
    )
    xs_in = nc.dram_tensor("xs", [128, NG, 4, 32], BF16, kind="ExternalInput")
    v_out = nc.dram_tensor("v", [B_LOC, J, D], F32, kind="ExternalOutput")

    # Selector constants for cross-partition PE ops:
    #   sel[p, b'] = 1 iff p % 4 == b'     (collapse the 32 (c,i8)-strips)
    #   sel32 = sel / J                    (fold in the uniform iter-0 softmax)
    #   selT[b, p] = sel^T                 (replicate over the 32 strips)
    p_idx = np.arange(128)
    sel_np = (p_idx[:, None] % B_LOC == np.arange(B_LOC)[None, :]).astype(np.float32)
    selpack = np.zeros((128, 2 * B_LOC + 128), np.float32)
    selpack[:, 0:B_LOC] = sel_np
    selpack[:, B_LOC : 2 * B_LOC] = sel_np / J
    selpack[0:B_LOC, 2 * B_LOC :] = sel_np.T
    sel_dram = nc.inline_tensor(selpack.astype(ml_dtypes.bfloat16), "selpack")

    with tile.TileContext(nc) as tc:
        with (
            tc.tile_pool(name="persist", bufs=1) as pp,
            tc.tile_pool(name="small", bufs=1) as sp,
            tc.tile_pool(name="spsum", bufs=1, space="PSUM") as ssp,
        ):
            # ---- persistent SBUF tensors ----
            xs = pp.tile([128, NG, 4, 32], BF16)        # block-diag x stationaries
            C = pp.tile([128, NG, 2, D, J], BF16)       # u_hat
            # logits in bf16: the per-block update runs at DVE 2x/4x instead
            # of 1x; rounding (~0.4% of logit scale) is far inside the error
            # budget (measured margin ~5x)
            bl = pp.tile([128, NG, 2, J], BF16)         # routing logits
            p_t = pp.tile([128, NG, 2, J], BF16)        # exp(b)
            selc = pp.tile([128, 2 * B_LOC + 128], BF16)
            v_rep = pp.tile([128, D, J], BF16)          # v replicated over strips

            sel = selc[:, 0:B_LOC]
            sel32 = selc[:, B_LOC : 2 * B_LOC]
            selT = selc[0:B_LOC, 2 * B_LOC :]

            # persistent PSUM: s accumulator + v/fac replication banks
            s_ps = ssp.tile([B_LOC, D * J], F32)
            vr_ps = ssp.tile([128, D * J], F32)
            fr_ps = ssp.tile([128, J], F32)

            # a tiny leading xs slice (g 0-1, 64 KB) unblocks the first
            # matmuls ~3us earlier than a monolithic xs transfer; the rest
            # streams behind it. selc trails (only needed once s0 starts).
            # No bl memset: iteration 0's logit update writes bl.
            for qs in (slice(0, 2), slice(2, 8), slice(8, 16),
                       slice(16, 24), slice(24, 32)):
                nc.gpsimd.dma_start(xs[:, qs], xs_in[:, qs])
            nc.scalar.dma_start(selc[:], sel_dram[:])

            # Warm the ACT exp table set at t~0 (under the W DMA shadow) so
            # the first softmax doesn't stall ~2.7us. The squash sqrt runs as
            # a DVE Newton rsqrt, so exp is the only table set ever loaded.
            wa = sp.tile([1, 8], F32, tag="wa")
            wb = sp.tile([1, 8], F32, tag="wb")
            nc.vector.memset(wa[:], 1.0)
            nc.scalar.activation(wb[:], wa[:], Act.Exp)

            # constants for the Newton rsqrt (quake seed)
            U32 = mybir.dt.uint32
            rc_magic = sp.tile([B_LOC, 1], U32, tag="rc_magic")
            rc_one = sp.tile([B_LOC, 1], U32, tag="rc_one")
            rc_15 = sp.tile([B_LOC, 1], F32, tag="rc_15")
            nc.vector.memset(rc_magic[:], 0x5F3759DF)
            nc.vector.memset(rc_one[:], 1)
            nc.vector.memset(rc_15[:], 1.5)

            # ---- phase 1: u_hat + iteration-0 s accumulation ----
            ns0 = [0]

            def s0_slice(g, rp):
                kk = ns0[0]
                ns0[0] += 1
                nc.tensor.matmul(
                    s_ps[:],
                    sel32,
                    C[:, g, rp].rearrange("p d j -> p (d j)"),
                    start=(kk == 0),
                    stop=(kk == 2 * NG - 1),
                )

            with (
                tc.tile_pool(name="wpool", bufs=5) as wp,
                tc.tile_pool(name="psum1", bufs=2, space="PSUM") as ps1,
            ):
                def w_dma(ch):
                    wt = wp.tile([128, CHUNK_G, 4, JD], FP8 if W_FP8 else BF16, tag="wt")
                    # alternate the two HWDGE rings so two chunks are always
                    # in flight; the head chunks are split per-g (each ring is
                    # FIFO, so a smaller first transfer completes sooner and
                    # un-gates the first matmuls/casts earlier)
                    eng = nc.sync if ch % 2 == 0 else nc.scalar
                    if ch < 2:
                        for gg in range(CHUNK_G):
                            eng.dma_start(
                                wt[:, gg : gg + 1], wt_in[ch, :, gg : gg + 1]
                            )
                    else:
                        eng.dma_start(wt[:], wt_in[ch])
                    return wt

                wts = [w_dma(0), w_dma(1), w_dma(2), w_dma(3)]
                nsub = 0
                for ch in range(NCHUNK):
                    wt = wts.pop(0)
                    if ch + 4 < NCHUNK:
                        wts.append(w_dma(ch + 4))
                    for gg in range(CHUNK_G):
                        g = ch * CHUNK_G + gg
                        pg = ps1.tile([128, 2, JD], F32, tag="pg")
                        for rp in range(2):
                            for c in range(4):
                                nc.tensor.matmul(
                                    pg[32 * c : 32 * c + 32, rp, :],
                                    xs[64 * rp : 64 * rp + 64, g, c, :],
                                    wt[64 * rp : 64 * rp + 64, gg, c, :],
                                    tile_position=(64 * rp, 32 * c),
                                )
                        dst = C[:, g].rearrange("p rp d j -> p rp (d j)")
                        # alternate casts DVE/ACT: doubles copy-chain
                        # throughput (the phase-1 tail binds on it); the
                        # 4-deep W prefetch absorbs the <=1us issue delay a
                        # copy can add ahead of a scalar-ring W DIRECT2D
                        if g % 2 == 1:
                            nc.scalar.copy(dst, pg[:])
                        else:
                            nc.vector.tensor_copy(dst, pg[:])
                        nsub += 1
                        if nsub > 2:
                            g2 = nsub - 3
                            s0_slice(g2, 0)
                            s0_slice(g2, 1)
                for g2 in range(NG - 2, NG):
                    s0_slice(g2, 0)
                    s0_slice(g2, 1)

            # ---- routing ----
            with tc.tile_pool(name="blk", bufs=2) as bp:
                for it in range(ROUTINGS):
                    # squash factor from s_ps (all on [B_LOC(=4), ...]);
                    # Square first: the n2 chain is the critical path, the
                    # s_gb copy only feeds the later replication matmul
                    sq = sp.tile([B_LOC, D, J], F32, tag="sq")
                    nc.scalar.activation(
                        sq.rearrange("b d j -> b (d j)"), s_ps[:], Act.Square
                    )
                    s_gb = sp.tile([B_LOC, D, J], BF16, tag="s_gb")
                    nc.scalar.copy(s_gb.rearrange("b d j -> b (d j)"), s_ps[:])
                    n2 = sp.tile([B_LOC, J], F32, tag="n2")
                    nc.vector.tensor_reduce(
                        n2[:],
                        sq.rearrange("b d j -> b j d"),
                        axis=mybir.AxisListType.X,
                        op=Alu.add,
                    )
                    # factor = n2 / (1 + n2) / sqrt(n2 + eps); rsqrt via a
                    # 2-step Newton iteration with the quake bitcast seed —
                    # all DVE, so the ACT engine never leaves the exp table
                    # set (a Sqrt/Ln activation would reload tables ~2.6us
                    # per iteration)
                    n2e = sp.tile([B_LOC, J], F32, tag="n2e")
                    nc.vector.tensor_scalar_add(n2e[:], n2[:], EPS)
                    hf = sp.tile([B_LOC, J], F32, tag="hf")
                    nc.vector.tensor_scalar_mul(hf[:], n2e[:], 0.5)
                    zi = sp.tile([B_LOC, J], U32, tag="zi")
                    nc.vector.tensor_tensor(
                        zi[:], n2e.bitcast(U32),
                        rc_one.broadcast_to([B_LOC, J]),
                        op=Alu.logical_shift_right,
                    )
                    ry = sp.tile([B_LOC, J], F32, tag="ry")
                    nc.vector.tensor_tensor(
                        ry.bitcast(U32), rc_magic.broadcast_to([B_LOC, J]),
                        zi[:], op=Alu.subtract,
                    )
                    rt = sp.tile([B_LOC, J], F32, tag="rt")
                    for _ in range(2):
                        nc.vector.tensor_tensor(rt[:], ry[:], ry[:], op=Alu.mult)
                        nc.vector.tensor_tensor(rt[:], rt[:], hf[:], op=Alu.mult)
                        nc.vector.tensor_tensor(
                            rt[:], rc_15.broadcast_to([B_LOC, J]), rt[:],
                            op=Alu.subtract,
                        )
                        nc.vector.tensor_tensor(ry[:], ry[:], rt[:], op=Alu.mult)
                    tmp = sp.tile([B_LOC, J], F32, tag="tmp")
                    nc.vector.tensor_scalar_add(tmp[:], n2[:], 1.0)
                    qr = sp.tile([B_LOC, J], F32, tag="qr")
                    nc.vector.reciprocal(qr[:], tmp[:])
                    fac = sp.tile([B_LOC, J], F32, tag="fac")
                    nc.vector.tensor_tensor(fac[:], qr[:], ry[:], op=Alu.mult)

                    if it < ROUTINGS - 1:
                        facb = sp.tile([B_LOC, J], BF16, tag="facb")
                        nc.vector.tensor_tensor(facb[:], fac[:], n2[:], op=Alu.mult)
                        # replicate s and fac over the 32 strips via the PE
                        nc.tensor.matmul(
                            vr_ps[:], selT, s_gb.rearrange("b d j -> b (d j)")
                        )
                        nc.tensor.matmul(fr_ps[:], selT, facb[:])
                        fr_sb = sp.tile([128, J], BF16, tag="fr_sb")
                        nc.scalar.copy(fr_sb[:], fr_ps[:])
                        nc.vector.tensor_tensor(
                            v_rep[:],
                            vr_ps.rearrange("p (d j) -> p d j", d=D, j=J),
                            fr_sb[:, None, :].broadcast_to([128, D, J]),
                            op=Alu.mult,
                        )
                        # fused per-block routing step. The j-softmax is
                        # slice-local, so each block runs its ENTIRE chain —
                        # agreement (log2 tree over d), logit update, exp,
                        # block-local softmax sum (folded into the s-matmul
                        # selector), coefficient product, s-matmuls — before
                        # the next block. The s-matmul chain (13.8 us serial
                        # on the PE) then overlaps the remaining blocks' DVE
                        # work instead of all queueing after pass A.
                        for blk in range(NBLK):
                            gs = slice(blk * GB, blk * GB + GB)
                            Cb = C[:, gs]
                            pi2 = bp.tile([128, GB, 2, D, J], BF16, tag="pi2")
                            nc.vector.tensor_tensor(
                                pi2[:],
                                Cb,
                                v_rep[:, None, None, :, :].broadcast_to(
                                    [128, GB, 2, D, J]
                                ),
                                op=Alu.mult,
                            )
                            t8 = bp.tile([128, GB, 2, 8, J], BF16, tag="t8")
                            nc.vector.tensor_tensor(
                                t8[:], pi2[:, :, :, 0:8, :], pi2[:, :, :, 8:16, :],
                                op=Alu.add,
                            )
                            t4 = bp.tile([128, GB, 2, 4, J], BF16, tag="t4")
                            nc.vector.tensor_tensor(
                                t4[:], t8[:, :, :, 0:4, :], t8[:, :, :, 4:8, :],
                                op=Alu.add,
                            )
                            t2 = bp.tile([128, GB, 2, 2, J], BF16, tag="t2")
                            nc.vector.tensor_tensor(
                                t2[:], t4[:, :, :, 0:2, :], t4[:, :, :, 2:4, :],
                                op=Alu.add,
                            )
                            t1 = bp.tile([128, GB, 2, J], BF16, tag="t1")
                            nc.vector.tensor_tensor(
                                t1[:], t2[:, :, :, 0, :], t2[:, :, :, 1, :],
                                op=Alu.add,
                            )
                            if it == 0:
                                nc.vector.tensor_copy(bl[:, gs], t1[:])
                            else:
                                nc.vector.tensor_add(bl[:, gs], bl[:, gs], t1[:])
                            nc.scalar.activation(p_t[:, gs], bl[:, gs], Act.Exp)
                            S = bp.tile([128, GB, 2], F32, tag="S")
                            nc.vector.tensor_reduce(
                                S[:], p_t[:, gs], axis=mybir.AxisListType.X,
                                op=Alu.add,
                            )
                            Sr = bp.tile([128, GB, 2], BF16, tag="Sr")
                            with nc.allow_low_precision(
                                reason="softmax divisor in bf16: c is consumed "
                                "in bf16 products anyway"
                            ):
                                nc.vector.reciprocal(Sr[:], S[:])
                            sel_s = bp.tile([128, GB, 2, B_LOC], BF16, tag="sel_s")
                            nc.vector.tensor_tensor(
                                sel_s[:],
                                sel[:, None, None, :].broadcast_to(
                                    [128, GB, 2, B_LOC]
                                ),
                                Sr[:, :, :, None].broadcast_to(
                                    [128, GB, 2, B_LOC]
                                ),
                                op=Alu.mult,
                            )
                            nsplit = 4 if blk == NBLK - 1 else 1
                            for sb in range(nsplit):
                                gw = GB // nsplit
                                goff = blk * GB + sb * gw
                                gsp = slice(goff, goff + gw)
                                pi = bp.tile([128, GB, 2, D, J], BF16, tag="pi")
                                piv = pi[:, 0:gw]
                                nc.vector.tensor_tensor(
                                    piv,
                                    C[:, gsp],
                                    p_t[:, gsp, :, None, :].broadcast_to(
                                        [128, gw, 2, D, J]
                                    ),
                                    op=Alu.mult,
                                )
                                for gr in range(gw * 2):
                                    g2, rp = divmod(gr, 2)
                                    kk = goff * 2 + gr
                                    nc.tensor.matmul(
                                        s_ps[:],
                                        sel_s[:, sb * gw + g2, rp, :],
                                        piv[:, g2, rp].rearrange(
                                            "p d j -> p (d j)"
                                        ),
                                        start=(kk == 0),
                                        stop=(kk == 2 * NG - 1),
                                    )
                    else:
                        # final output: v = s * fac in f32, (d, j) -> (j, d)
                        facf = sp.tile([B_LOC, J], F32, tag="facf")
                        nc.vector.tensor_tensor(
                            facf[:], fac[:], n2[:], op=Alu.mult
                        )
                        v_jd = sp.tile([B_LOC, J, D], F32, tag="v_jd")
                        nc.vector.tensor_tensor(
                            v_jd[:],
                            s_gb.rearrange("b d j -> b j d"),
                            facf[:, :, None].broadcast_to([B_LOC, J, D]),
                            op=Alu.mult,
                        )
                        nc.sync.dma_start(v_out[:], v_jd[:])

    nc.compile()
    return nc


def _prep_inputs(x, W):
    """Host-side layout prep (bf16). W is shared by all cores; x is B-sliced."""
    # wt[ch, p=(rp,i8,k), gg, c, (d j)] = W[j, i, d, k],
    # i = ((g*2+rp)*4+c)*8 + i8, g = ch*CHUNK_G + gg
    Wr = W.reshape(J, NG, 2, 4, 8, D, K)                  # j g rp c i8 d k
    wt = np.ascontiguousarray(Wr.transpose(1, 2, 4, 6, 3, 5, 0))  # g rp i8 k c d j
    wt = wt.reshape(NCHUNK, CHUNK_G, 128, 4, JD).transpose(0, 2, 1, 3, 4)
    if W_FP8:
        wt = np.ascontiguousarray(wt * W_SCALE).astype(ml_dtypes.float8_e4m3)
    else:
        wt = np.ascontiguousarray(wt).astype(ml_dtypes.bfloat16)
    i8 = np.arange(8)
    if W_FP8:
        x = x / W_SCALE  # exact in bf16 (power-of-2); undoes the W scale
    in_maps = []
    for m in range(NCORES):
        xb = x[B_LOC * m : B_LOC * (m + 1)].reshape(B_LOC, NG, 2, 4, 8, K)
        xsz = np.zeros((2, 8, K, NG, 4, 8, B_LOC), np.float32)  # rp i8 k g c i8' b
        xsz[:, i8, :, :, :, i8, :] = xb.transpose(4, 2, 5, 1, 3, 0)[i8]
        in_maps.append(
            {
                "wt": wt,
                "xs": xsz.reshape(128, NG, 4, 32).astype(ml_dtypes.bfloat16),
            }
        )
    return in_maps


def run(inputs, trace=False):
    if "nc" not in _CACHE:
        _CACHE["nc"] = _build()
    nc = _CACHE["nc"]
    in_maps = _prep_inputs(np.asarray(inputs["x"]), np.asarray(inputs["W"]))
    bkr = run_bass_kernel_spmd(
        nc, in_maps, core_ids=list(range(NCORES)), trace=trace
    )
    out = np.concatenate(
        [bkr.results[m]["v"].astype(np.float32) for m in range(NCORES)], axis=0
    )
    return out, bkr


def kernel(x, W):
    out, _ = run({"x": np.asarray(x), "W": np.asarray(W)})
    return out
